# revision 31
# baseline (speedup 1.0000x reference)
"""Trainium2 Bass kernel for nn_KITRO (gnn_message_passing).

Pure data parallel over 8 NeuronCores: batch 8192 -> 1024 per core.

Phase A (depth MLP) runs in fp8-e4m3 with DoubleRow (double-pumped)
matmuls on the PE:
  features f32 --SWDGE casting DMA--> X8 fp8 SBUF (batch-natural)
  --xbar 2-byte-pair transpose (SBUF->SBUF)--> XT fp8 with feature pairs
  (2p, 2p+1) interleaved per partition p, which is exactly the
  [K, 2, N] layout DoubleRow matmuls consume (weight rows deinterleaved
  host-side to match).  Per-layer scales are chosen so every relu
  epilogue is a single (add bias, max 0) tensor_scalar at scale 1:
    h1q = relu(S1*W1^T x + S1*b1)            (S1 = 8,  stored fp8)
    h2q = relu(S2*W2^T h1q + S1*S2*b2)       (S2 = 4,  stored fp8)
    d   = (S3*W3^T h2q) / (S3*S1*S2) + b3    (S3 = 64, stored f32)
  plus zT = (SZ*0.5*cW1[4:])^T x / SZ in bf16 (bone feature projection,
  averaged later in 32-dim z space -- algebraically exact).
  Epilogues rotate across ACT / DVE / GpSimd so no single engine binds.

Phase B (3 bone refinement iterations): batch-on-partition geometry on
DVE/ACT, tiny bone MLP via block-diagonal bf16 matmuls in transposed
layout on PE, per-batch mean over bones fused into the last matmul.
"""

import sys

if "/opt/trn_rl_repo" not in sys.path:
    sys.path.insert(0, "/opt/trn_rl_repo")

import numpy as np

import concourse.bass as bass
import concourse.mybir as mybir
import concourse.tile as tile
from concourse import bacc
from concourse.bass_utils import run_bass_kernel_spmd
from concourse.masks import make_identity

F32 = mybir.dt.float32
BF16 = mybir.dt.bfloat16
FP8 = mybir.dt.float8e4
AF = mybir.ActivationFunctionType
OP = mybir.AluOpType
DR = mybir.MatmulPerfMode.DoubleRow

NCORE = 8
B = 8192
BC = B // NCORE          # 1024 batches per core
J = 25
FD = 512
HD = 1024
ROWS = BC * J            # 25600 rows per core
RC = 512                 # row-chunk
NCH = ROWS // RC         # 50 chunks

S1 = 8.0                 # W1 & h1 scale
S2 = 4.0                 # W2 scale; h2 stored at S1*S2 = 32
S3 = 64.0                # W3 scale
SZ = 64.0                # z-projection scale

_CACHE = {}

import os as _os
# NOTE: issuing xbar transposes concurrently on both HWDGE rings corrupts
# data (shared crossbar state) -- all transposes go on the sync ring.
BUFS4 = _os.environ.get("K_BUFS4", "1") == "1"      # deeper prefetch
EARLY2D = _os.environ.get("K_EARLY2D", "1") == "1"  # pos2d copy in phase A
HALF_AVGZ = _os.environ.get("K_HALFAVGZ", "1") == "1"  # avgz inside phase A
DRAM_XPOSE = _os.environ.get("K_DRAMX", "1") == "1"  # cast->DRAM, fast xpose


def _build_nc():
    nc = bacc.Bacc("TRN2", target_bir_lowering=False, debug=False,
                   num_devices=NCORE)

    # ---- per-core DRAM I/O ----
    feats = nc.dram_tensor("features", [BC, J, FD], F32, kind="ExternalInput")
    p2d = nc.dram_tensor("poses_2d", [BC, J, 2], F32, kind="ExternalInput")
    conf = nc.dram_tensor("confidence", [BC, J], F32, kind="ExternalInput")
    # pre-scaled / reordered weights (host-prepared)
    # layouts keep each DoubleRow weight pair contiguous: [.., 2, 128]
    w1r = nc.dram_tensor("w1r", [128, 2, 8, 2, 128], F32,
                         kind="ExternalInput")      # [p, fb, mt, i, mi]
    wzr = nc.dram_tensor("wzr", [128, 2, 2, 32], F32, kind="ExternalInput")
    w2r = nc.dram_tensor("w2r", [128, 4, 4, 2, 128], F32,
                         kind="ExternalInput")      # [p, ip, nt, j, ni]
    w3r = nc.dram_tensor("w3r", [128, 2, 2, 1], F32, kind="ExternalInput")
    b1x = nc.dram_tensor("b1x", [HD], F32, kind="ExternalInput")
    b2x = nc.dram_tensor("b2x", [FD], F32, kind="ExternalInput")
    db3 = nc.dram_tensor("db3", [1], F32, kind="ExternalInput")
    cW1a = nc.dram_tensor("cW1a", [4, 32], F32, kind="ExternalInput")
    cb1 = nc.dram_tensor("cb1", [32], F32, kind="ExternalInput")
    cW2 = nc.dram_tensor("cW2", [32, 64], F32, kind="ExternalInput")
    cb2 = nc.dram_tensor("cb2", [64], F32, kind="ExternalInput")
    cW3 = nc.dram_tensor("cW3", [64, 3], F32, kind="ExternalInput")
    cb3x = nc.dram_tensor("cb3x", [3], F32, kind="ExternalInput")  # 0.1*cb3
    out = nc.dram_tensor("out", [BC, J, 3], F32, kind="ExternalOutput")

    (feats, p2d, conf, w1r, wzr, w2r, w3r, b1x, b2x, db3,
     cW1a, cb1, cW2, cb2, cW3, cb3x, out) = (
        t.ap() for t in (feats, p2d, conf, w1r, wzr, w2r, w3r, b1x, b2x, db3,
                         cW1a, cb1, cW2, cb2, cW3, cb3x, out))

    feats_flat = feats.flatten_outer_dims()          # [ROWS, FD]

    with tile.TileContext(nc) as tc:
        import contextlib
        with contextlib.ExitStack() as ctx:
            const = ctx.enter_context(tc.tile_pool(name="const", bufs=1))
            dram = ctx.enter_context(
                tc.tile_pool(name="dram", bufs=1, space="DRAM"))

            # ---- constants / weights ----
            id_bf = const.tile([128, 128], BF16, tag="id")
            make_identity(nc, id_bf)
            id3 = const.tile([3, 3], F32, tag="id3")
            make_identity(nc, id3)

            # fp8 phase-A weights via casting SWDGE loads
            w1q = const.tile([128, 2, 8, 2, 128], FP8, tag="w1q")
            nc.gpsimd.dma_start(out=w1q, in_=w1r)
            wzq = const.tile([128, 2, 2, 32], FP8, tag="wzq")
            nc.gpsimd.dma_start(out=wzq, in_=wzr)
            w2q = const.tile([128, 4, 4, 2, 128], FP8, tag="w2q")
            nc.gpsimd.dma_start(out=w2q, in_=w2r)
            w3q = const.tile([128, 2, 2, 1], FP8, tag="w3q")
            nc.gpsimd.dma_start(out=w3q, in_=w3r)

            # block-diagonal bone weights (bf16, phase B)
            blkW1 = const.tile([64, 512], BF16, tag="blkW1")   # 16x cW1[:4]
            nc.vector.memset(blkW1, 0.0)
            for d in range(16):
                nc.gpsimd.dma_start(
                    out=blkW1[4 * d:4 * d + 4, 32 * d:32 * d + 32],
                    in_=cW1a)
            blkW2 = const.tile([128, 256], BF16, tag="blkW2")  # 4x cW2
            nc.vector.memset(blkW2, 0.0)
            for d in range(4):
                nc.gpsimd.dma_start(
                    out=blkW2[32 * d:32 * d + 32, 64 * d:64 * d + 64],
                    in_=cW2[:, :])
            w3stk = const.tile([128, 3], BF16, tag="w3stk")    # cW3 stacked 2x
            for d in range(2):
                nc.gpsimd.dma_start(out=w3stk[64 * d:64 * d + 64, :],
                                    in_=cW3[:, :])

            # biases
            db1_sb = const.tile([128, 8], F32, tag="db1")
            nc.sync.dma_start(out=db1_sb,
                              in_=b1x.rearrange("(m p) -> p m", p=128))
            db2_sb = const.tile([128, 4], F32, tag="db2")
            nc.sync.dma_start(out=db2_sb,
                              in_=b2x.rearrange("(m p) -> p m", p=128))
            db3_sb = const.tile([1, 1], F32, tag="db3")
            nc.sync.dma_start(out=db3_sb,
                              in_=db3.rearrange("(a o) -> a o", a=1))
            cb1_sb = const.tile([128, 1], F32, tag="cb1")
            for q in range(4):
                nc.sync.dma_start(out=cb1_sb[32 * q:32 * q + 32, :],
                                  in_=cb1.rearrange("(m o) -> m o", o=1))
            cb2_sb = const.tile([128, 1], F32, tag="cb2")
            for q in range(2):
                nc.sync.dma_start(out=cb2_sb[64 * q:64 * q + 64, :],
                                  in_=cb2.rearrange("(m o) -> m o", o=1))
            cb3s = const.tile([3, 1], F32, tag="cb3")
            nc.sync.dma_start(out=cb3s,
                              in_=cb3x.rearrange("(m o) -> m o", o=1))

            # persistent activations
            zT = const.tile([32, ROWS], BF16, tag="zT")        # [32, (b j)]
            pos3 = const.tile([128, 8, J, 3], F32, tag="pos3")
            conf_b = const.tile([128, 8, J], F32, tag="conf")
            avgz = const.tile([32, 16, BC], BF16, tag="avgz")
            y1h = [const.tile([128, BC], BF16, tag=f"y1h{q}", name=f"y1h{q}")
                   for q in range(4)]

            # phase-B inputs: load early (ACT HWDGE ring), overlap phase A
            p2d_b = const.tile([128, 8, J, 2], F32, tag="p2db")
            nc.scalar.dma_start(
                out=p2d_b.rearrange("p bh j c -> p bh (j c)"),
                in_=p2d.rearrange("(bh bl) j c -> bl bh (j c)", bl=128))
            nc.scalar.dma_start(
                out=conf_b,
                in_=conf.rearrange("(bh bl) j -> bl bh j", bl=128))

            # DRAM scratch for depths + fp8-cast features
            dscr = dram.tile([ROWS], F32)
            xcD = dram.tile([ROWS, FD], FP8)

            # ---------------- Phase A: depth MLP (fp8) ----------------
            # gpsimd cannot read PSUM, so epilogues rotate ACT/DVE only
            EW = ["A", "D", "A", "D", "A", "D", "A", "D",   # h1 m=0..7
                  "A", "D", "A", "A"]                       # h2 n=0..3

            # bone adjacency groups (child slice, parent slice)
            groups = [(0, 6, slice(1, 7), slice(0, 1)),
                      (6, 8, slice(7, 9), slice(5, 7)),
                      (8, 10, slice(9, 11), slice(7, 9)),
                      (10, 12, slice(11, 13), slice(5, 7)),
                      (12, 14, slice(13, 15), slice(11, 13)),
                      (14, 16, slice(15, 17), slice(13, 15))]
            zv = zT.rearrange("p (b j) -> p j b", j=J)     # [32, J, BC]

            def avgz_half(h, nh=2):
                """avgz + y1h scatter for batches [h*BC/nh, (h+1)*BC/nh)."""
                w = BC // nh
                bs_ = slice(h * w, (h + 1) * w)
                for (e0, e1, cs, ps_) in groups:
                    n = e1 - e0
                    in1 = zv[:, ps_, bs_]
                    if in1.shape[1] != n:
                        in1 = in1.to_broadcast([32, n, w])
                    nc.vector.tensor_tensor(out=avgz[:, e0:e1, bs_],
                                            in0=zv[:, cs, bs_], in1=in1,
                                            op=OP.add)
                for e in range(16):
                    q, s_ = e // 4, e % 4
                    eng = nc.sync if e % 2 == 0 else nc.scalar
                    eng.dma_start(out=y1h[q][32 * s_:32 * s_ + 32, bs_],
                                  in_=avgz[:, e, bs_])

            def epilogue(which, out_ap, ps, bias_ap):
                if which == "A":
                    nc.scalar.activation(out=out_ap, in_=ps, func=AF.Relu,
                                         bias=bias_ap)
                else:
                    nc.vector.tensor_scalar(out=out_ap, in0=ps,
                                            scalar1=bias_ap, scalar2=0.0,
                                            op0=OP.add, op1=OP.max)

            nbuf = 4 if BUFS4 else 3
            with tc.tile_pool(name="x8p", bufs=nbuf) as x8p, \
                 tc.tile_pool(name="xtp", bufs=nbuf) as xtp, \
                 tc.tile_pool(name="h1p", bufs=2) as h1p, \
                 tc.tile_pool(name="h2p", bufs=2) as h2p, \
                 tc.tile_pool(name="dscp", bufs=3) as dscp, \
                 tc.tile_pool(name="psA", bufs=4, space="PSUM") as psA, \
                 tc.tile_pool(name="psZ", bufs=2, space="PSUM") as psZ, \
                 tc.tile_pool(name="psD", bufs=2, space="PSUM") as psD:

                xcDv = xcD.bitcast(BF16)            # [ROWS, 256]
                for c in range(NCH):
                    rs = slice(c * RC, (c + 1) * RC)
                    xt = xtp.tile([128, 2, 2 * RC], FP8, tag="xt")
                    xtv = xt.bitcast(BF16)          # [128, 2, 512]
                    if DRAM_XPOSE:
                        # fp32 -> fp8 cast DRAM->DRAM (SWDGE), then 2-byte-
                        # pair xbar transposes DRAM->SBUF on the sync ring
                        nc.gpsimd.dma_start(out=xcD[rs, :],
                                            in_=feats_flat[rs])
                        for fb in range(2):
                            nc.sync.dma_start_transpose(
                                xtv[:, fb, :],
                                xcDv[rs, fb * 128:(fb + 1) * 128])
                    else:
                        x8 = x8p.tile([128, 4, RC], FP8, tag="x8")
                        nc.gpsimd.dma_start(
                            out=x8,
                            in_=feats_flat[rs].rearrange(
                                "(rg p) f -> p rg f", p=128))
                        x8v = x8.bitcast(BF16)      # [128, 4, 256]
                        for rg in range(4):
                            nc.sync.dma_start_transpose(
                                xtv[:, :, rg * 128:(rg + 1) * 128],
                                x8v[:, rg, :])

                    def xt_rhs(fb):
                        return xt[:, fb, :].rearrange("p (r i) -> p i r", i=2)

                    # W1 (+bias+relu -> h1q fp8)
                    h1q = h1p.tile([128, 8, RC], FP8, tag="h1")
                    for m in range(8):
                        ps = psA.tile([128, RC], F32, tag="mm")
                        for fb in range(2):
                            nc.tensor.matmul(
                                ps, w1q[:, fb, m],
                                xt_rhs(fb), start=(fb == 0), stop=(fb == 1),
                                perf_mode=DR)
                        epilogue(EW[m], h1q[:, m, :], ps, db1_sb[:, m:m + 1])

                    # z projection (shares XT)
                    psz = psZ.tile([32, RC], F32, tag="z")
                    for fb in range(2):
                        nc.tensor.matmul(psz, wzq[:, fb], xt_rhs(fb),
                                         start=(fb == 0), stop=(fb == 1),
                                         perf_mode=DR)
                    nc.vector.tensor_scalar_mul(zT[:, rs], psz, 1.0 / SZ)

                    # W2 (+bias+relu -> h2q fp8)
                    h2q = h2p.tile([128, 4, RC], FP8, tag="h2")
                    for n in range(4):
                        ps = psA.tile([128, RC], F32, tag="mm")
                        for i2 in range(4):
                            nc.tensor.matmul(
                                ps, w2q[:, i2, n],
                                h1q[:, 2 * i2:2 * i2 + 2, :],
                                start=(i2 == 0), stop=(i2 == 3),
                                perf_mode=DR)
                        epilogue(EW[8 + n], h2q[:, n, :], ps,
                                 db2_sb[:, n:n + 1])

                    # W3 -> depths
                    # W3 stationary is 1 column -- too narrow for dual-fp8
                    # ldweights, so plain fp8 matmuls (4 K-tiles)
                    psd = psD.tile([1, RC], F32, tag="d")
                    for kh in range(4):
                        nc.tensor.matmul(psd, w3q[:, kh // 2, kh % 2],
                                         h2q[:, kh, :],
                                         start=(kh == 0), stop=(kh == 3))
                    dsc = dscp.tile([1, RC], F32, tag="dsc")
                    nc.vector.tensor_scalar(out=dsc, in0=psd,
                                            scalar1=1.0 / (S3 * S1 * S2),
                                            scalar2=db3_sb[0:1, 0:1],
                                            op0=OP.mult, op1=OP.add)
                    nc.sync.dma_start(out=dscr[rs], in_=dsc)

                    if c == 2 and EARLY2D:
                        # overlap phase-B pose init with phase A
                        nc.vector.tensor_copy(out=pos3[:, :, :, 0:2],
                                              in_=p2d_b)
                    elif c == NCH // 2 and HALF_AVGZ:
                        avgz_half(0)
                if HALF_AVGZ:
                    avgz_half(1)

            # ---------------- Phase B: bone refinement ----------------
            dep_b = const.tile([128, 8, J], F32, tag="depb")
            nc.scalar.dma_start(
                out=dep_b,
                in_=dscr.rearrange("(bh bl j) -> bl bh j", bl=128, j=J))
            nc.vector.tensor_copy(out=pos3[:, :, :, 2], in_=dep_b)
            if not EARLY2D:
                nc.vector.tensor_copy(out=pos3[:, :, :, 0:2], in_=p2d_b)
            if not HALF_AVGZ:
                avgz_half(0, nh=1)

            with tc.tile_pool(name="pb", bufs=2) as pb, \
                 tc.tile_pool(name="g1p", bufs=2) as g1p, \
                 tc.tile_pool(name="g2p", bufs=2) as g2p, \
                 tc.tile_pool(name="psB", bufs=3, space="PSUM") as psB, \
                 tc.tile_pool(name="psPU", bufs=1, space="PSUM") as psPU, \
                 tc.tile_pool(name="psTR", bufs=2, space="PSUM") as psTR:

                for it in range(3):
                    # ---- geometry (batch-on-partition, fp32) ----
                    bv = pb.tile([128, 8, 16, 3], F32, tag="bv")
                    for (e0, e1, cs, ps_) in groups:
                        n = e1 - e0
                        in1 = pos3[:, :, ps_, :]
                        if in1.shape[2] != n:
                            in1 = in1.to_broadcast([128, 8, n, 3])
                        nc.vector.tensor_tensor(out=bv[:, :, e0:e1, :],
                                                in0=pos3[:, :, cs, :], in1=in1,
                                                op=OP.subtract)
                    sq = pb.tile([128, 8, 16, 3], F32, tag="sq")
                    nc.vector.tensor_tensor(out=sq, in0=bv, in1=bv, op=OP.mult)
                    lensq = pb.tile([128, 8, 16], F32, tag="lensq")
                    nc.vector.tensor_reduce(out=lensq, in_=sq,
                                            axis=mybir.AxisListType.X,
                                            op=OP.add)
                    dl = pb.tile([128, 8, 16, 4], BF16, tag="dl")
                    nc.scalar.activation(out=dl[:, :, :, 3], in_=lensq,
                                         func=AF.Sqrt)
                    inv = pb.tile([128, 8, 16], F32, tag="inv")
                    nc.vector.tensor_scalar(out=inv, in0=dl[:, :, :, 3],
                                            scalar1=1e-8, scalar2=None,
                                            op0=OP.add)
                    nc.vector.reciprocal(inv, inv)
                    nc.vector.tensor_tensor(
                        out=dl[:, :, :, 0:3], in0=bv,
                        in1=inv[:, :, :, None].to_broadcast([128, 8, 16, 3]),
                        op=OP.mult)

                    # transpose dirlen to [(e,4), b] layout
                    dlT = pb.tile([64, 8, 128], BF16, tag="dlT")
                    for bh in range(8):
                        pst = psTR.tile([128, 128], BF16, tag="tr")
                        nc.tensor.transpose(pst[:64, :], dl[:, bh], id_bf)
                        nc.vector.tensor_copy(out=dlT[:, bh, :],
                                              in_=pst[:64, :])

                    # ---- bone MLP (transposed layout) ----
                    g1 = [g1p.tile([128, 8, 128], BF16, tag=f"g1_{q}",
                                   name=f"g1_{q}_{it}")
                          for q in range(4)]
                    for q in range(4):
                        for hh in range(2):
                            bs = slice(hh * 4, hh * 4 + 4)
                            ps = psB.tile([128, RC], F32, tag="mm")
                            nc.tensor.matmul(
                                ps, blkW1[:, 128 * q:128 * q + 128],
                                dlT[:, bs, :],
                                start=True, stop=False)
                            nc.tensor.matmul(
                                ps, id_bf, y1h[q][:, hh * 512:hh * 512 + 512],
                                start=False, stop=True)
                            nc.scalar.activation(out=g1[q][:, bs, :], in_=ps,
                                                 func=AF.Relu, bias=cb1_sb)
                    g2 = [g2p.tile([128, 8, 128], BF16, tag=f"g2_{c2}",
                                   name=f"g2_{c2}_{it}")
                          for c2 in range(8)]
                    for c2 in range(8):
                        q, half = c2 // 2, c2 % 2
                        for hh in range(2):
                            bs = slice(hh * 4, hh * 4 + 4)
                            ps = psB.tile([128, RC], F32, tag="mm")
                            nc.tensor.matmul(
                                ps, blkW2[:, 128 * half:128 * half + 128],
                                g1[q][:, bs, :],
                                start=True, stop=True)
                            nc.scalar.activation(out=g2[c2][:, bs, :], in_=ps,
                                                 func=AF.Relu, bias=cb2_sb)
                    pu_sb = pb.tile([3, 8, 128], F32, tag="pu_sb")
                    for hh in range(2):
                        bs = slice(hh * 4, hh * 4 + 4)
                        psu = psPU.tile([3, RC], F32, tag="pu")
                        for c2 in range(8):
                            nc.tensor.matmul(psu, w3stk, g2[c2][:, bs, :],
                                             start=(c2 == 0), stop=(c2 == 7))
                        # 0.1 * (sum/16 + cb3) = sum*(0.1/16) + 0.1*cb3
                        nc.scalar.activation(out=pu_sb[:, bs, :], in_=psu,
                                             func=AF.Identity, bias=cb3s,
                                             scale=0.1 / 16.0)
                    # transpose update back to batch-on-partition layout
                    pu_b = pb.tile([128, 8, 3], F32, tag="pu_b")
                    for bh in range(8):
                        pst = psTR.tile([128, 4], F32, tag="trpu")
                        nc.tensor.transpose(pst[:, :3], pu_sb[:, bh, :],
                                            id3)
                        nc.vector.tensor_copy(out=pu_b[:, bh, :],
                                              in_=pst[:, :3])
                    # pos3 = (pos3 + pu) * conf
                    nc.vector.tensor_tensor(
                        out=pos3, in0=pos3,
                        in1=pu_b[:, :, None, :].to_broadcast([128, 8, J, 3]),
                        op=OP.add)
                    nc.vector.tensor_tensor(
                        out=pos3, in0=pos3,
                        in1=conf_b[:, :, :, None].to_broadcast([128, 8, J, 3]),
                        op=OP.mult)

            nc.sync.dma_start(
                out=out.rearrange("(bh bl) j c -> bl bh j c", bl=128),
                in_=pos3)

    nc.compile()
    return nc


def _get_nc():
    if "nc" not in _CACHE:
        _CACHE["nc"] = _build_nc()
    return _CACHE["nc"]


def _prep_weights(inputs):
    """Host-side scale + reorder of the (tiny) MLP weights."""
    f32 = np.float32
    dW1 = np.asarray(inputs["dW1"], f32)
    dW2 = np.asarray(inputs["dW2"], f32)
    dW3 = np.asarray(inputs["dW3"], f32)
    cW1 = np.asarray(inputs["cW1"], f32)
    # w1r[p, fb, mt, i, mi] = S1 * dW1[fb*256 + 2p + i, mt*128 + mi]
    w1r = np.ascontiguousarray(
        (dW1 * S1).reshape(2, 128, 2, 8, 128).transpose(1, 0, 3, 2, 4))
    wzr = np.ascontiguousarray(
        (0.5 * SZ * cW1[4:]).reshape(2, 128, 2, 32).transpose(1, 0, 2, 3))
    # w2r[p, ip, nt, j, ni] = S2 * dW2[(2*ip+j)*128 + p, nt*128 + ni]
    w2r = np.ascontiguousarray(
        (dW2 * S2).reshape(4, 2, 128, 4, 128).transpose(2, 0, 3, 1, 4))
    # w3r[p, ip, j, 0] = S3 * dW3[(2*ip+j)*128 + p, 0]
    w3r = np.ascontiguousarray(
        (dW3 * S3).reshape(2, 2, 128, 1).transpose(2, 0, 1, 3))
    return {
        "w1r": w1r, "wzr": wzr, "w2r": w2r, "w3r": w3r,
        "b1x": np.asarray(inputs["db1"], f32) * S1,
        "b2x": np.asarray(inputs["db2"], f32) * (S1 * S2),
        "db3": np.asarray(inputs["db3"], f32),
        "cW1a": np.ascontiguousarray(cW1[:4]),
        "cb1": np.asarray(inputs["cb1"], f32),
        "cW2": np.asarray(inputs["cW2"], f32),
        "cb2": np.asarray(inputs["cb2"], f32),
        "cW3": np.asarray(inputs["cW3"], f32),
        "cb3x": np.asarray(inputs["cb3"], f32) * 0.1,
    }


def _in_maps(inputs):
    wmap = _prep_weights(inputs)
    maps = []
    for c in range(NCORE):
        bs = slice(c * BC, (c + 1) * BC)
        m = {
            "features": np.ascontiguousarray(inputs["features"][bs]),
            "poses_2d": np.ascontiguousarray(inputs["poses_2d"][bs]),
            "confidence": np.ascontiguousarray(inputs["confidence"][bs]),
        }
        m.update(wmap)
        maps.append(m)
    return maps


def _run(inputs, **kw):
    nc = _get_nc()
    res = run_bass_kernel_spmd(nc, _in_maps(inputs),
                               core_ids=list(range(NCORE)), **kw)
    full = np.concatenate([res.results[c]["out"] for c in range(NCORE)],
                          axis=0)
    return full.astype(np.float32), res


def kernel(**inputs) -> np.ndarray:
    out, _ = _run(inputs)
    return out


# revision 34
# speedup vs baseline: 1.1437x; 1.1437x over previous
"""Trainium2 Bass kernel for nn_KITRO (gnn_message_passing).

Pure data parallel over 8 NeuronCores: batch 8192 -> 1024 per core.

Phase A (depth MLP) runs in fp8-e4m3 with DoubleRow (double-pumped)
matmuls on the PE:
  features f32 --SWDGE casting DMA--> X8 fp8 SBUF (batch-natural)
  --xbar 2-byte-pair transpose (SBUF->SBUF)--> XT fp8 with feature pairs
  (2p, 2p+1) interleaved per partition p, which is exactly the
  [K, 2, N] layout DoubleRow matmuls consume (weight rows deinterleaved
  host-side to match).  Per-layer scales are chosen so every relu
  epilogue is a single (add bias, max 0) tensor_scalar at scale 1:
    h1q = relu(S1*W1^T x + S1*b1)            (S1 = 8,  stored fp8)
    h2q = relu(S2*W2^T h1q + S1*S2*b2)       (S2 = 4,  stored fp8)
    d   = (S3*W3^T h2q) / (S3*S1*S2) + b3    (S3 = 64, stored f32)
  plus zT = (SZ*0.5*cW1[4:])^T x / SZ in bf16 (bone feature projection,
  averaged later in 32-dim z space -- algebraically exact).
  Epilogues rotate across ACT / DVE / GpSimd so no single engine binds.

Phase B (3 bone refinement iterations): batch-on-partition geometry on
DVE/ACT, tiny bone MLP via block-diagonal bf16 matmuls in transposed
layout on PE, per-batch mean over bones fused into the last matmul.
"""

import sys

if "/opt/trn_rl_repo" not in sys.path:
    sys.path.insert(0, "/opt/trn_rl_repo")

import numpy as np

import concourse.bass as bass
import concourse.mybir as mybir
import concourse.tile as tile
from concourse import bacc
from concourse.bass_utils import run_bass_kernel_spmd
from concourse.masks import make_identity

F32 = mybir.dt.float32
BF16 = mybir.dt.bfloat16
FP8 = mybir.dt.float8e4
AF = mybir.ActivationFunctionType
OP = mybir.AluOpType
DR = mybir.MatmulPerfMode.DoubleRow

NCORE = 8
B = 8192
BC = B // NCORE          # 1024 batches per core
J = 25
FD = 512
HD = 1024
ROWS = BC * J            # 25600 rows per core
RC = 512                 # row-chunk
NCH = ROWS // RC         # 50 chunks

S1 = 8.0                 # W1 & h1 scale
S2 = 4.0                 # W2 scale; h2 stored at S1*S2 = 32
S3 = 64.0                # W3 scale
SZ = 64.0                # z-projection scale

_CACHE = {}

import os as _os
# NOTE: issuing xbar transposes concurrently on both HWDGE rings corrupts
# data (shared crossbar state) -- all transposes go on the sync ring.
BUFS4 = _os.environ.get("K_BUFS4", "1") == "1"      # deeper prefetch
EARLY2D = _os.environ.get("K_EARLY2D", "1") == "1"  # pos2d copy in phase A
HALF_AVGZ = _os.environ.get("K_HALFAVGZ", "1") == "1"  # avgz inside phase A
DRAM_XPOSE = _os.environ.get("K_DRAMX", "1") == "1"  # cast->DRAM, fast xpose


def _build_nc():
    nc = bacc.Bacc("TRN2", target_bir_lowering=False, debug=False,
                   num_devices=NCORE)

    # ---- per-core DRAM I/O ----
    feats = nc.dram_tensor("features", [BC, J, FD], F32, kind="ExternalInput")
    p2d = nc.dram_tensor("poses_2d", [BC, J, 2], F32, kind="ExternalInput")
    conf = nc.dram_tensor("confidence", [BC, J], F32, kind="ExternalInput")
    # pre-scaled / reordered weights (host-prepared)
    # layouts keep each DoubleRow weight pair contiguous: [.., 2, 128]
    w1r = nc.dram_tensor("w1r", [128, 2, 8, 2, 128], F32,
                         kind="ExternalInput")      # [p, fb, mt, i, mi]
    wzr = nc.dram_tensor("wzr", [128, 2, 2, 32], F32, kind="ExternalInput")
    w2r = nc.dram_tensor("w2r", [128, 4, 4, 2, 128], F32,
                         kind="ExternalInput")      # [p, ip, nt, j, ni]
    w3r = nc.dram_tensor("w3r", [128, 2, 2, 1], F32, kind="ExternalInput")
    b1x = nc.dram_tensor("b1x", [HD], F32, kind="ExternalInput")
    b2x = nc.dram_tensor("b2x", [FD], F32, kind="ExternalInput")
    db3 = nc.dram_tensor("db3", [1], F32, kind="ExternalInput")
    cW1a = nc.dram_tensor("cW1a", [4, 32], F32, kind="ExternalInput")
    cb1 = nc.dram_tensor("cb1", [32], F32, kind="ExternalInput")
    cW2 = nc.dram_tensor("cW2", [32, 64], F32, kind="ExternalInput")
    cb2 = nc.dram_tensor("cb2", [64], F32, kind="ExternalInput")
    cW3 = nc.dram_tensor("cW3", [64, 3], F32, kind="ExternalInput")
    cb3x = nc.dram_tensor("cb3x", [3], F32, kind="ExternalInput")  # 0.1*cb3
    out = nc.dram_tensor("out", [BC, J, 3], F32, kind="ExternalOutput")

    (feats, p2d, conf, w1r, wzr, w2r, w3r, b1x, b2x, db3,
     cW1a, cb1, cW2, cb2, cW3, cb3x, out) = (
        t.ap() for t in (feats, p2d, conf, w1r, wzr, w2r, w3r, b1x, b2x, db3,
                         cW1a, cb1, cW2, cb2, cW3, cb3x, out))

    feats_flat = feats.flatten_outer_dims()          # [ROWS, FD]

    with tile.TileContext(nc) as tc:
        import contextlib
        with contextlib.ExitStack() as ctx:
            const = ctx.enter_context(tc.tile_pool(name="const", bufs=1))
            dram = ctx.enter_context(
                tc.tile_pool(name="dram", bufs=1, space="DRAM"))

            # ---- constants / weights ----
            id_bf = const.tile([128, 128], BF16, tag="id")
            make_identity(nc, id_bf)
            id3 = const.tile([3, 3], F32, tag="id3")
            make_identity(nc, id3)

            # fp8 phase-A weights via casting SWDGE loads
            w1q = const.tile([128, 2, 8, 2, 128], FP8, tag="w1q")
            nc.gpsimd.dma_start(out=w1q, in_=w1r)
            wzq = const.tile([128, 2, 2, 32], FP8, tag="wzq")
            nc.gpsimd.dma_start(out=wzq, in_=wzr)
            w2q = const.tile([128, 4, 4, 2, 128], FP8, tag="w2q")
            nc.gpsimd.dma_start(out=w2q, in_=w2r)
            w3q = const.tile([128, 2, 2, 1], FP8, tag="w3q")
            nc.gpsimd.dma_start(out=w3q, in_=w3r)

            # biases
            db1_sb = const.tile([128, 8], F32, tag="db1")
            nc.sync.dma_start(out=db1_sb,
                              in_=b1x.rearrange("(m p) -> p m", p=128))
            db2_sb = const.tile([128, 4], F32, tag="db2")
            nc.sync.dma_start(out=db2_sb,
                              in_=b2x.rearrange("(m p) -> p m", p=128))
            db3_sb = const.tile([1, 1], F32, tag="db3")
            nc.sync.dma_start(out=db3_sb,
                              in_=db3.rearrange("(a o) -> a o", a=1))
            cb1_sb = const.tile([128, 1], F32, tag="cb1")
            for q in range(4):
                nc.sync.dma_start(out=cb1_sb[32 * q:32 * q + 32, :],
                                  in_=cb1.rearrange("(m o) -> m o", o=1))
            cb2_sb = const.tile([128, 1], F32, tag="cb2")
            for q in range(2):
                nc.sync.dma_start(out=cb2_sb[64 * q:64 * q + 64, :],
                                  in_=cb2.rearrange("(m o) -> m o", o=1))
            cb3s = const.tile([3, 1], F32, tag="cb3")
            nc.sync.dma_start(out=cb3s,
                              in_=cb3x.rearrange("(m o) -> m o", o=1))

            # persistent activations
            zT = const.tile([32, ROWS], BF16, tag="zT")        # [32, (b j)]
            pos3 = const.tile([128, 8, J, 3], F32, tag="pos3")
            conf_b = const.tile([128, 8, J], F32, tag="conf")
            avgz = const.tile([32, 16, BC], BF16, tag="avgz")
            y1h = [const.tile([128, BC], BF16, tag=f"y1h{q}", name=f"y1h{q}")
                   for q in range(4)]

            # phase-B inputs: load early (ACT HWDGE ring), overlap phase A
            p2d_b = const.tile([128, 8, J, 2], F32, tag="p2db")
            nc.scalar.dma_start(
                out=p2d_b.rearrange("p bh j c -> p bh (j c)"),
                in_=p2d.rearrange("(bh bl) j c -> bl bh (j c)", bl=128))
            nc.scalar.dma_start(
                out=conf_b,
                in_=conf.rearrange("(bh bl) j -> bl bh j", bl=128))

            # DRAM scratch for depths + fp8-cast features
            dscr = dram.tile([ROWS], F32)
            xcD = dram.tile([ROWS, FD], FP8)

            # ---------------- Phase A: depth MLP (fp8) ----------------
            # gpsimd cannot read PSUM, so epilogues rotate ACT/DVE only
            EW = ["A", "D", "A", "D", "A", "D", "A", "D",   # h1 m=0..7
                  "A", "D", "A", "A"]                       # h2 n=0..3

            # bone adjacency groups (child slice, parent slice)
            groups = [(0, 6, slice(1, 7), slice(0, 1)),
                      (6, 8, slice(7, 9), slice(5, 7)),
                      (8, 10, slice(9, 11), slice(7, 9)),
                      (10, 12, slice(11, 13), slice(5, 7)),
                      (12, 14, slice(13, 15), slice(11, 13)),
                      (14, 16, slice(15, 17), slice(13, 15))]
            zv = zT.rearrange("p (b j) -> p j b", j=J)     # [32, J, BC]

            def avgz_half(h, nh=2):
                """avgz + y1h scatter for batches [h*BC/nh, (h+1)*BC/nh)."""
                w = BC // nh
                bs_ = slice(h * w, (h + 1) * w)
                for (e0, e1, cs, ps_) in groups:
                    n = e1 - e0
                    in1 = zv[:, ps_, bs_]
                    if in1.shape[1] != n:
                        in1 = in1.to_broadcast([32, n, w])
                    nc.vector.tensor_tensor(out=avgz[:, e0:e1, bs_],
                                            in0=zv[:, cs, bs_], in1=in1,
                                            op=OP.add)
                for e in range(16):
                    q, s_ = e // 4, e % 4
                    eng = nc.sync if e % 2 == 0 else nc.scalar
                    eng.dma_start(out=y1h[q][32 * s_:32 * s_ + 32, bs_],
                                  in_=avgz[:, e, bs_])

            def epilogue(which, out_ap, ps, bias_ap):
                if which == "A":
                    nc.scalar.activation(out=out_ap, in_=ps, func=AF.Relu,
                                         bias=bias_ap)
                else:
                    nc.vector.tensor_scalar(out=out_ap, in0=ps,
                                            scalar1=bias_ap, scalar2=0.0,
                                            op0=OP.add, op1=OP.max)

            nbuf = (6 if DRAM_XPOSE else 4) if BUFS4 else 3
            with tc.tile_pool(name="x8p", bufs=3) as x8p, \
                 tc.tile_pool(name="xtp", bufs=nbuf) as xtp, \
                 tc.tile_pool(name="h1p", bufs=2) as h1p, \
                 tc.tile_pool(name="h2p", bufs=2) as h2p, \
                 tc.tile_pool(name="dscp", bufs=3) as dscp, \
                 tc.tile_pool(name="psA", bufs=4, space="PSUM") as psA, \
                 tc.tile_pool(name="psZ", bufs=2, space="PSUM") as psZ, \
                 tc.tile_pool(name="psD", bufs=2, space="PSUM") as psD:

                xcDv = xcD.bitcast(BF16)            # [ROWS, 256]
                for c in range(NCH):
                    rs = slice(c * RC, (c + 1) * RC)
                    xt = xtp.tile([128, 2, 2 * RC], FP8, tag="xt")
                    xtv = xt.bitcast(BF16)          # [128, 2, 512]
                    if DRAM_XPOSE:
                        # fp32 -> fp8 cast DRAM->DRAM (SWDGE), then 2-byte-
                        # pair xbar transposes DRAM->SBUF on the sync ring
                        nc.gpsimd.dma_start(out=xcD[rs, :],
                                            in_=feats_flat[rs])
                        for fb in range(2):
                            nc.sync.dma_start_transpose(
                                xtv[:, fb, :],
                                xcDv[rs, fb * 128:(fb + 1) * 128])
                    else:
                        x8 = x8p.tile([128, 4, RC], FP8, tag="x8")
                        nc.gpsimd.dma_start(
                            out=x8,
                            in_=feats_flat[rs].rearrange(
                                "(rg p) f -> p rg f", p=128))
                        x8v = x8.bitcast(BF16)      # [128, 4, 256]
                        for rg in range(4):
                            nc.sync.dma_start_transpose(
                                xtv[:, :, rg * 128:(rg + 1) * 128],
                                x8v[:, rg, :])

                    def xt_rhs(fb):
                        return xt[:, fb, :].rearrange("p (r i) -> p i r", i=2)

                    # W1 (+bias+relu -> h1q fp8)
                    h1q = h1p.tile([128, 8, RC], FP8, tag="h1")
                    for m in range(8):
                        ps = psA.tile([128, RC], F32, tag="mm")
                        for fb in range(2):
                            nc.tensor.matmul(
                                ps, w1q[:, fb, m],
                                xt_rhs(fb), start=(fb == 0), stop=(fb == 1),
                                perf_mode=DR)
                        epilogue(EW[m], h1q[:, m, :], ps, db1_sb[:, m:m + 1])

                    # z projection (shares XT)
                    psz = psZ.tile([32, RC], F32, tag="z")
                    for fb in range(2):
                        nc.tensor.matmul(psz, wzq[:, fb], xt_rhs(fb),
                                         start=(fb == 0), stop=(fb == 1),
                                         perf_mode=DR)
                    nc.vector.tensor_scalar_mul(zT[:, rs], psz, 1.0 / SZ)

                    # W2 (+bias+relu -> h2q fp8)
                    h2q = h2p.tile([128, 4, RC], FP8, tag="h2")
                    for n in range(4):
                        ps = psA.tile([128, RC], F32, tag="mm")
                        for i2 in range(4):
                            nc.tensor.matmul(
                                ps, w2q[:, i2, n],
                                h1q[:, 2 * i2:2 * i2 + 2, :],
                                start=(i2 == 0), stop=(i2 == 3),
                                perf_mode=DR)
                        epilogue(EW[8 + n], h2q[:, n, :], ps,
                                 db2_sb[:, n:n + 1])

                    # W3 -> depths
                    # W3 stationary is 1 column -- too narrow for dual-fp8
                    # ldweights, so plain fp8 matmuls (4 K-tiles)
                    psd = psD.tile([1, RC], F32, tag="d")
                    for kh in range(4):
                        nc.tensor.matmul(psd, w3q[:, kh // 2, kh % 2],
                                         h2q[:, kh, :],
                                         start=(kh == 0), stop=(kh == 3))
                    dsc = dscp.tile([1, RC], F32, tag="dsc")
                    nc.vector.tensor_scalar(out=dsc, in0=psd,
                                            scalar1=1.0 / (S3 * S1 * S2),
                                            scalar2=db3_sb[0:1, 0:1],
                                            op0=OP.mult, op1=OP.add)
                    nc.sync.dma_start(out=dscr[rs], in_=dsc)

                    if c == 2 and EARLY2D:
                        # overlap phase-B pose init with phase A
                        nc.vector.tensor_copy(out=pos3[:, :, :, 0:2],
                                              in_=p2d_b)
                    elif c == NCH // 2 and HALF_AVGZ:
                        avgz_half(0)
                if HALF_AVGZ:
                    avgz_half(1)

            # ---------------- Phase B: bone refinement ----------------
            # block-diagonal bone weights (bf16) -- issued after the phase-A
            # chunk stream so their SWDGE loads don't delay feature casts
            blkW1 = const.tile([64, 512], BF16, tag="blkW1")   # 16x cW1[:4]
            nc.vector.memset(blkW1, 0.0)
            for d in range(16):
                nc.gpsimd.dma_start(
                    out=blkW1[4 * d:4 * d + 4, 32 * d:32 * d + 32],
                    in_=cW1a)
            blkW2 = const.tile([128, 256], BF16, tag="blkW2")  # 4x cW2
            nc.vector.memset(blkW2, 0.0)
            for d in range(4):
                nc.gpsimd.dma_start(
                    out=blkW2[32 * d:32 * d + 32, 64 * d:64 * d + 64],
                    in_=cW2[:, :])
            w3stk = const.tile([128, 3], BF16, tag="w3stk")    # cW3 stacked 2x
            for d in range(2):
                nc.gpsimd.dma_start(out=w3stk[64 * d:64 * d + 64, :],
                                    in_=cW3[:, :])

            dep_b = const.tile([128, 8, J], F32, tag="depb")
            nc.scalar.dma_start(
                out=dep_b,
                in_=dscr.rearrange("(bh bl j) -> bl bh j", bl=128, j=J))
            nc.vector.tensor_copy(out=pos3[:, :, :, 2], in_=dep_b)
            if not EARLY2D:
                nc.vector.tensor_copy(out=pos3[:, :, :, 0:2], in_=p2d_b)
            if not HALF_AVGZ:
                avgz_half(0, nh=1)

            with tc.tile_pool(name="pb", bufs=2) as pb, \
                 tc.tile_pool(name="g1p", bufs=2) as g1p, \
                 tc.tile_pool(name="g2p", bufs=2) as g2p, \
                 tc.tile_pool(name="psB", bufs=3, space="PSUM") as psB, \
                 tc.tile_pool(name="psPU", bufs=1, space="PSUM") as psPU, \
                 tc.tile_pool(name="psTR", bufs=2, space="PSUM") as psTR:

                for it in range(3):
                    # ---- geometry (batch-on-partition, fp32) ----
                    bv = pb.tile([128, 8, 16, 3], F32, tag="bv")
                    for (e0, e1, cs, ps_) in groups:
                        n = e1 - e0
                        in1 = pos3[:, :, ps_, :]
                        if in1.shape[2] != n:
                            in1 = in1.to_broadcast([128, 8, n, 3])
                        nc.vector.tensor_tensor(out=bv[:, :, e0:e1, :],
                                                in0=pos3[:, :, cs, :], in1=in1,
                                                op=OP.subtract)
                    sq = pb.tile([128, 8, 16, 3], F32, tag="sq")
                    nc.vector.tensor_tensor(out=sq, in0=bv, in1=bv, op=OP.mult)
                    lensq = pb.tile([128, 8, 16], F32, tag="lensq")
                    nc.vector.tensor_reduce(out=lensq, in_=sq,
                                            axis=mybir.AxisListType.X,
                                            op=OP.add)
                    dl = pb.tile([128, 8, 16, 4], BF16, tag="dl")
                    nc.scalar.activation(out=dl[:, :, :, 3], in_=lensq,
                                         func=AF.Sqrt)
                    inv = pb.tile([128, 8, 16], F32, tag="inv")
                    nc.vector.tensor_scalar(out=inv, in0=dl[:, :, :, 3],
                                            scalar1=1e-8, scalar2=None,
                                            op0=OP.add)
                    nc.vector.reciprocal(inv, inv)
                    nc.vector.tensor_tensor(
                        out=dl[:, :, :, 0:3], in0=bv,
                        in1=inv[:, :, :, None].to_broadcast([128, 8, 16, 3]),
                        op=OP.mult)

                    # transpose dirlen to [(e,4), b] layout
                    dlT = pb.tile([64, 8, 128], BF16, tag="dlT")
                    for bh in range(8):
                        pst = psTR.tile([128, 128], BF16, tag="tr")
                        nc.tensor.transpose(pst[:64, :], dl[:, bh], id_bf)
                        nc.vector.tensor_copy(out=dlT[:, bh, :],
                                              in_=pst[:64, :])

                    # ---- bone MLP (transposed layout) ----
                    g1 = [g1p.tile([128, 8, 128], BF16, tag=f"g1_{q}",
                                   name=f"g1_{q}_{it}")
                          for q in range(4)]
                    for q in range(4):
                        for hh in range(2):
                            bs = slice(hh * 4, hh * 4 + 4)
                            ps = psB.tile([128, RC], F32, tag="mm")
                            nc.tensor.matmul(
                                ps, blkW1[:, 128 * q:128 * q + 128],
                                dlT[:, bs, :],
                                start=True, stop=False)
                            nc.tensor.matmul(
                                ps, id_bf, y1h[q][:, hh * 512:hh * 512 + 512],
                                start=False, stop=True)
                            nc.scalar.activation(out=g1[q][:, bs, :], in_=ps,
                                                 func=AF.Relu, bias=cb1_sb)
                    g2 = [g2p.tile([128, 8, 128], BF16, tag=f"g2_{c2}",
                                   name=f"g2_{c2}_{it}")
                          for c2 in range(8)]
                    for c2 in range(8):
                        q, half = c2 // 2, c2 % 2
                        for hh in range(2):
                            bs = slice(hh * 4, hh * 4 + 4)
                            ps = psB.tile([128, RC], F32, tag="mm")
                            nc.tensor.matmul(
                                ps, blkW2[:, 128 * half:128 * half + 128],
                                g1[q][:, bs, :],
                                start=True, stop=True)
                            nc.scalar.activation(out=g2[c2][:, bs, :], in_=ps,
                                                 func=AF.Relu, bias=cb2_sb)
                    pu_sb = pb.tile([3, 8, 128], F32, tag="pu_sb")
                    for hh in range(2):
                        bs = slice(hh * 4, hh * 4 + 4)
                        psu = psPU.tile([3, RC], F32, tag="pu")
                        for c2 in range(8):
                            nc.tensor.matmul(psu, w3stk, g2[c2][:, bs, :],
                                             start=(c2 == 0), stop=(c2 == 7))
                        # 0.1 * (sum/16 + cb3) = sum*(0.1/16) + 0.1*cb3
                        nc.scalar.activation(out=pu_sb[:, bs, :], in_=psu,
                                             func=AF.Identity, bias=cb3s,
                                             scale=0.1 / 16.0)
                    # transpose update back to batch-on-partition layout
                    pu_b = pb.tile([128, 8, 3], F32, tag="pu_b")
                    for bh in range(8):
                        pst = psTR.tile([128, 4], F32, tag="trpu")
                        nc.tensor.transpose(pst[:, :3], pu_sb[:, bh, :],
                                            id3)
                        nc.vector.tensor_copy(out=pu_b[:, bh, :],
                                              in_=pst[:, :3])
                    # pos3 = (pos3 + pu) * conf
                    nc.vector.tensor_tensor(
                        out=pos3, in0=pos3,
                        in1=pu_b[:, :, None, :].to_broadcast([128, 8, J, 3]),
                        op=OP.add)
                    nc.vector.tensor_tensor(
                        out=pos3, in0=pos3,
                        in1=conf_b[:, :, :, None].to_broadcast([128, 8, J, 3]),
                        op=OP.mult)

            nc.sync.dma_start(
                out=out.rearrange("(bh bl) j c -> bl bh j c", bl=128),
                in_=pos3)

    nc.compile()
    return nc


def _get_nc():
    if "nc" not in _CACHE:
        _CACHE["nc"] = _build_nc()
    return _CACHE["nc"]


def _prep_weights(inputs):
    """Host-side scale + reorder of the (tiny) MLP weights."""
    f32 = np.float32
    dW1 = np.asarray(inputs["dW1"], f32)
    dW2 = np.asarray(inputs["dW2"], f32)
    dW3 = np.asarray(inputs["dW3"], f32)
    cW1 = np.asarray(inputs["cW1"], f32)
    # w1r[p, fb, mt, i, mi] = S1 * dW1[fb*256 + 2p + i, mt*128 + mi]
    w1r = np.ascontiguousarray(
        (dW1 * S1).reshape(2, 128, 2, 8, 128).transpose(1, 0, 3, 2, 4))
    wzr = np.ascontiguousarray(
        (0.5 * SZ * cW1[4:]).reshape(2, 128, 2, 32).transpose(1, 0, 2, 3))
    # w2r[p, ip, nt, j, ni] = S2 * dW2[(2*ip+j)*128 + p, nt*128 + ni]
    w2r = np.ascontiguousarray(
        (dW2 * S2).reshape(4, 2, 128, 4, 128).transpose(2, 0, 3, 1, 4))
    # w3r[p, ip, j, 0] = S3 * dW3[(2*ip+j)*128 + p, 0]
    w3r = np.ascontiguousarray(
        (dW3 * S3).reshape(2, 2, 128, 1).transpose(2, 0, 1, 3))
    return {
        "w1r": w1r, "wzr": wzr, "w2r": w2r, "w3r": w3r,
        "b1x": np.asarray(inputs["db1"], f32) * S1,
        "b2x": np.asarray(inputs["db2"], f32) * (S1 * S2),
        "db3": np.asarray(inputs["db3"], f32),
        "cW1a": np.ascontiguousarray(cW1[:4]),
        "cb1": np.asarray(inputs["cb1"], f32),
        "cW2": np.asarray(inputs["cW2"], f32),
        "cb2": np.asarray(inputs["cb2"], f32),
        "cW3": np.asarray(inputs["cW3"], f32),
        "cb3x": np.asarray(inputs["cb3"], f32) * 0.1,
    }


def _in_maps(inputs):
    wmap = _prep_weights(inputs)
    maps = []
    for c in range(NCORE):
        bs = slice(c * BC, (c + 1) * BC)
        m = {
            "features": np.ascontiguousarray(inputs["features"][bs]),
            "poses_2d": np.ascontiguousarray(inputs["poses_2d"][bs]),
            "confidence": np.ascontiguousarray(inputs["confidence"][bs]),
        }
        m.update(wmap)
        maps.append(m)
    return maps


def _run(inputs, **kw):
    nc = _get_nc()
    res = run_bass_kernel_spmd(nc, _in_maps(inputs),
                               core_ids=list(range(NCORE)), **kw)
    full = np.concatenate([res.results[c]["out"] for c in range(NCORE)],
                          axis=0)
    return full.astype(np.float32), res


def kernel(**inputs) -> np.ndarray:
    out, _ = _run(inputs)
    return out
